# revision 9
# baseline (speedup 1.0000x reference)
"""MoE (top-2 of 16 experts, SwiGLU MLP) kernel for 8 Trainium2 NeuronCores.

Strategy (expert-parallel, per sharding hint):
  - Host: router (x @ w_gate -> softmax -> top-2) computed in float64,
    tokens gathered per expert ("all-to-all"), padded to a uniform
    capacity C (multiple of 128).
  - Device (SPMD over 8 cores, 2 experts/core): per expert
        ht = silu(W1e.T @ Xt) * (W2e.T @ Xt)     [feature-major layout]
        yt = (WCe.T @ ht) * gate_row
    All matmuls run as float32r (full PE rate, ~1e-4 rel err),
    accumulating fp32 in PSUM.
  - Host: scatter-add per-expert outputs back to token order.
"""

import contextlib
import ctypes
import os
import sys
import types

sys.path.insert(0, "/opt/trn_rl_repo")

import numpy as np

import concourse.bass as bass
import concourse.mybir as mybir
import concourse.tile as tile

EMB = 1024
HID = 1024
E = 16
TOPK = 2
NCORES = 8
EPC = E // NCORES  # experts per core
P = 128
F32 = None  # set after mybir import below
F32R = None


def _install_profile_shim():
    """Register the axon NTFF profiling hook (missing antenv.axon_hooks in
    this image) so run_bass_kernel_spmd(trace=True) can measure HW time."""
    if "antenv.axon_hooks" in sys.modules:
        return
    try:
        lib = ctypes.CDLL("/opt/axon/libaxon_pjrt.so")
        lib.axon_start_nrt_profile.argtypes = [
            ctypes.POINTER(ctypes.c_int64),
            ctypes.c_size_t,
        ]
        lib.axon_start_nrt_profile.restype = ctypes.c_int64
        lib.axon_stop_nrt_profile.argtypes = [ctypes.c_char_p]
        lib.axon_stop_nrt_profile.restype = ctypes.c_int64
    except Exception:
        return

    @contextlib.contextmanager
    def _hook(output_dir, device_ids):
        import jax

        jax.devices()
        ids = (
            (ctypes.c_int64 * len(device_ids))(*device_ids) if device_ids else None
        )
        rc = lib.axon_start_nrt_profile(ids, len(device_ids) if device_ids else 0)
        if rc != 0:
            raise RuntimeError(f"axon_start_nrt_profile rc={rc}")
        try:
            yield
        finally:
            n = lib.axon_stop_nrt_profile(str(output_dir).encode())
            print(f"profile: {n} file(s) written to {output_dir}")

    mod = types.ModuleType("antenv.axon_hooks")
    mod.get_axon_ntff_profile_hook = lambda: _hook
    mod.set_axon_ntff_profile_hook = lambda h: None
    sys.modules["antenv.axon_hooks"] = mod


def _split_multi_waits(nc):
    """This container's walrus only encodes one sem wait per CTRL-class
    instruction; hoist extra waits onto dedicated single-wait NoOps."""
    idx = 0
    for fn in nc.m.functions:
        for bb in fn.blocks:
            new = []
            for inst in bb.instructions:
                si = inst.sync_info
                if si is not None and len(si.on_wait) > 1:
                    waits = list(si.on_wait)
                    for w in waits[:-1]:
                        c = mybir.InstNoOp(name=f"wsplit-{idx}", ins=[], outs=[])
                        idx += 1
                        c.engine = inst.engine
                        c.sync_info = mybir.SyncInfo(on_wait=[w], on_update=[])
                        new.append(c)
                    si.on_wait = [waits[-1]]
                new.append(inst)
            bb.instructions = new


def _token_chunks(C):
    """Split C (multiple of 128) into chunks <=512, each a multiple of 128,
    preferring >=256 (float32r full-rate threshold)."""
    n = max(1, -(-C // 512))
    tiles = C // P
    base, rem = divmod(tiles, n)
    sizes = [(base + (1 if i < rem else 0)) * P for i in range(n)]
    return [s for s in sizes if s > 0]


def _build_bass(caps):
    F32 = mybir.dt.float32
    F32R = mybir.dt.float32r
    KT = EMB // P  # contraction tiles (8)
    HT = HID // P  # hidden row-blocks (8)

    nc = bass.Bass()
    xt_d, g_d, w1_d, w2_d, wc_d, yt_d = [], [], [], [], [], []
    for e in range(EPC):
        C = caps[e]
        xt_d.append(nc.declare_dram_parameter(f"xt{e}", [EMB, C], F32R, isOutput=False))
        g_d.append(nc.declare_dram_parameter(f"g{e}", [P, C], F32, isOutput=False))
        w1_d.append(
            nc.declare_dram_parameter(f"w1_{e}", [EMB, HID], F32R, isOutput=False)
        )
        w2_d.append(
            nc.declare_dram_parameter(f"w2_{e}", [EMB, HID], F32R, isOutput=False)
        )
        wc_d.append(
            nc.declare_dram_parameter(f"wc_{e}", [HID, EMB], F32R, isOutput=False)
        )
        yt_d.append(nc.declare_dram_parameter(f"yt{e}", [EMB, C], F32, isOutput=True))

    with tile.TileContext(nc) as tc:
        with (
            tc.tile_pool(name="xt", bufs=2) as xt_pool,
            tc.tile_pool(name="ht", bufs=1) as ht_pool,
            tc.tile_pool(name="g", bufs=2) as g_pool,
            tc.tile_pool(name="w12", bufs=4) as w12_pool,
            tc.tile_pool(name="wc", bufs=4) as wc_pool,
            tc.tile_pool(name="s", bufs=3) as s_pool,
            tc.tile_pool(name="y", bufs=3) as y_pool,
            tc.tile_pool(name="psA", bufs=2, space="PSUM") as psA,
            tc.tile_pool(name="psB", bufs=2, space="PSUM") as psB,
        ):
            for e in range(EPC):
                C = caps[e]
                chunks = _token_chunks(C)
                # h=0 weights + first xt chunk gate the first matmuls; issue
                # them per-k and interleaved so the k=0 pieces land first
                # instead of queueing behind bulk input DMA.
                w1t0 = w12_pool.tile([P, KT, P], mybir.dt.float32r, tag="w1")
                w2t0 = w12_pool.tile([P, KT, P], mybir.dt.float32r, tag="w2")
                w1v = w1_d[e][:, 0:P].rearrange("(k p) q -> p k q", p=P)
                w2v = w2_d[e][:, 0:P].rearrange("(k p) q -> p k q", p=P)
                xt_sb = xt_pool.tile([P, KT, C], mybir.dt.float32r, tag="xt")
                xt_view = xt_d[e].rearrange("(k p) c -> p k c", p=P)
                cs0 = chunks[0]
                for k in range(KT):
                    nc.sync.dma_start(w1t0[:, k, :], w1v[:, k, :])
                    nc.sync.dma_start(xt_sb[:, k, 0:cs0], xt_view[:, k, 0:cs0])
                for k in range(KT):
                    nc.sync.dma_start(w2t0[:, k, :], w2v[:, k, :])
                c0 = cs0
                for cs in chunks[1:]:
                    for k in range(KT):
                        nc.sync.dma_start(
                            xt_sb[:, k, c0 : c0 + cs],
                            xt_view[:, k, c0 : c0 + cs],
                        )
                    c0 += cs
                g_sb = g_pool.tile([P, C], mybir.dt.float32, tag="g")
                nc.sync.dma_start(g_sb[:], g_d[e][:])
                ht_sb = ht_pool.tile([P, HT, C], mybir.dt.float32r, tag="ht")

                # Phase A: ht = silu(W1.T @ Xt) * (W2.T @ Xt)
                for h in range(HT):
                    if h == 0:
                        w1t, w2t = w1t0, w2t0
                    else:
                        w1t = w12_pool.tile([P, KT, P], mybir.dt.float32r, tag="w1")
                        w2t = w12_pool.tile([P, KT, P], mybir.dt.float32r, tag="w2")
                        nc.sync.dma_start(
                            w1t[:],
                            w1_d[e][:, h * P : (h + 1) * P].rearrange(
                                "(k p) q -> p k q", p=P
                            ),
                        )
                        nc.sync.dma_start(
                            w2t[:],
                            w2_d[e][:, h * P : (h + 1) * P].rearrange(
                                "(k p) q -> p k q", p=P
                            ),
                        )
                    c0 = 0
                    for cs in chunks:
                        ps1 = psA.tile([P, cs], mybir.dt.float32, tag="ps1")
                        ps2 = psA.tile([P, cs], mybir.dt.float32, tag="ps2")
                        for k in range(KT):
                            nc.tensor.matmul(
                                ps1[:],
                                w1t[:, k, :],
                                xt_sb[:, k, c0 : c0 + cs],
                                start=(k == 0),
                                stop=(k == KT - 1),
                            )
                        for k in range(KT):
                            nc.tensor.matmul(
                                ps2[:],
                                w2t[:, k, :],
                                xt_sb[:, k, c0 : c0 + cs],
                                start=(k == 0),
                                stop=(k == KT - 1),
                            )
                        s_sb = s_pool.tile([P, 512], mybir.dt.float32, tag="s")
                        nc.scalar.activation(
                            s_sb[:, :cs],
                            ps1[:],
                            mybir.ActivationFunctionType.Silu,
                        )
                        nc.vector.tensor_mul(
                            ht_sb[:, h, c0 : c0 + cs], s_sb[:, :cs], ps2[:]
                        )
                        c0 += cs

                # Phase B: yt = (WC.T @ ht) * gate
                for d in range(HT):
                    wct = wc_pool.tile([P, HT, P], mybir.dt.float32r, tag="wc")
                    nc.sync.dma_start(
                        wct[:],
                        wc_d[e][:, d * P : (d + 1) * P].rearrange(
                            "(k p) q -> p k q", p=P
                        ),
                    )
                    c0 = 0
                    for cs in chunks:
                        psy = psB.tile([P, cs], mybir.dt.float32, tag="psy")
                        for h in range(HT):
                            nc.tensor.matmul(
                                psy[:],
                                wct[:, h, :],
                                ht_sb[:, h, c0 : c0 + cs],
                                start=(h == 0),
                                stop=(h == HT - 1),
                            )
                        y_sb = y_pool.tile([P, 512], mybir.dt.float32, tag="y")
                        nc.vector.tensor_mul(
                            y_sb[:, :cs], psy[:], g_sb[:, c0 : c0 + cs]
                        )
                        nc.sync.dma_start(
                            yt_d[e][d * P : (d + 1) * P, c0 : c0 + cs],
                            y_sb[:, :cs],
                        )
                        c0 += cs

    _split_multi_waits(nc)
    return nc


def kernel(x, w_gate, w1, w2, wc):
    trace = bool(int(os.environ.get("BASS_MOE_TRACE", "0")))
    if trace:
        _install_profile_shim()

    import concourse.bass_utils as bass_utils

    bass_utils.upload_artifacts = lambda tmpdir: f"local://{tmpdir}"

    x = np.asarray(x, dtype=np.float32)
    w_gate = np.asarray(w_gate, dtype=np.float32)
    w1 = np.asarray(w1, dtype=np.float32)
    w2 = np.asarray(w2, dtype=np.float32)
    wc = np.asarray(wc, dtype=np.float32)

    b, s, d = x.shape
    xf = x.reshape(-1, d)
    n = xf.shape[0]

    # ---- Router on host (float64: stable ranking + gate values) ----
    logits = xf.astype(np.float64) @ w_gate.astype(np.float64)
    mx = logits.max(axis=1, keepdims=True)
    p = np.exp(logits - mx)
    p /= p.sum(axis=1, keepdims=True)
    top = np.argpartition(-logits, TOPK, axis=1)[:, :TOPK]  # top-2 ids (unordered)

    sel_tok = []  # per expert: token indices
    sel_gate = []  # per expert: gate values
    flat_e = top.ravel()
    flat_t = np.repeat(np.arange(n), TOPK)
    order = np.argsort(flat_e, kind="stable")
    se, st = flat_e[order], flat_t[order]
    bounds = np.searchsorted(se, np.arange(E + 1))
    counts = np.diff(bounds)
    for e in range(E):
        toks = st[bounds[e] : bounds[e + 1]]
        sel_tok.append(toks)
        sel_gate.append(p[toks, e].astype(np.float32))

    # ---- Slot assignment: biggest experts in slot 0, smallest in slot 1,
    # so each slot's uniform capacity hugs its experts' actual counts ----
    rank = np.argsort(-counts, kind="stable")
    slot_experts = [
        [int(rank[core + j * NCORES]) for j in range(EPC)] for core in range(NCORES)
    ]
    caps = []
    for j in range(EPC):
        cmax = max(counts[slot_experts[core][j]] for core in range(NCORES))
        caps.append(max(P, int(-(-cmax // P) * P)))

    # ---- Build per-core input maps (expert-parallel: 2 experts/core) ----
    in_maps = []
    for core in range(NCORES):
        m = {}
        for j in range(EPC):
            e = slot_experts[core][j]
            C = caps[j]
            toks = sel_tok[e]
            xt = np.zeros((EMB, C), dtype=np.float32)
            xt[:, : len(toks)] = xf[toks].T
            g = np.zeros((C,), dtype=np.float32)
            g[: len(toks)] = sel_gate[e]
            m[f"xt{j}"] = xt
            m[f"g{j}"] = np.broadcast_to(g, (P, C)).copy()
            m[f"w1_{j}"] = np.ascontiguousarray(w1[e])
            m[f"w2_{j}"] = np.ascontiguousarray(w2[e])
            m[f"wc_{j}"] = np.ascontiguousarray(wc[e])
        in_maps.append(m)

    nc = _build_bass(caps)
    res = bass_utils.run_bass_kernel_spmd(
        nc, in_maps, list(range(NCORES)), trace=trace
    )
    if trace:
        kernel.last_exec_time_ns = res.exec_time_ns
        kernel.last_trace = (
            res.instructions_and_trace[1] if res.instructions_and_trace else None
        )

    # ---- Scatter-add back to token order ----
    out = np.zeros((n, d), dtype=np.float32)
    for core in range(NCORES):
        for j in range(EPC):
            e = slot_experts[core][j]
            toks = sel_tok[e]
            yt = res.results[core][f"yt{j}"]
            out[toks] += yt[:, : len(toks)].T
    return out.reshape(b, s, d)


# revision 10
# speedup vs baseline: 1.0349x; 1.0349x over previous
"""MoE (top-2 of 16 experts, SwiGLU MLP) kernel for 8 Trainium2 NeuronCores.

Strategy (expert-parallel, per sharding hint):
  - Host: router (x @ w_gate -> softmax -> top-2) computed in float64,
    tokens gathered per expert ("all-to-all"), padded to a uniform
    capacity C (multiple of 128).
  - Device (SPMD over 8 cores, 2 experts/core): per expert
        ht = silu(W1e.T @ Xt) * (W2e.T @ Xt)     [feature-major layout]
        yt = (WCe.T @ ht) * gate_row
    All matmuls run as float32r (full PE rate, ~1e-4 rel err),
    accumulating fp32 in PSUM.
  - Host: scatter-add per-expert outputs back to token order.
"""

import contextlib
import ctypes
import os
import sys
import types

sys.path.insert(0, "/opt/trn_rl_repo")

import numpy as np

import concourse.bass as bass
import concourse.mybir as mybir
import concourse.tile as tile

EMB = 1024
HID = 1024
E = 16
TOPK = 2
NCORES = 8
EPC = E // NCORES  # experts per core
P = 128
F32 = None  # set after mybir import below
F32R = None


def _install_profile_shim():
    """Register the axon NTFF profiling hook (missing antenv.axon_hooks in
    this image) so run_bass_kernel_spmd(trace=True) can measure HW time."""
    if "antenv.axon_hooks" in sys.modules:
        return
    try:
        lib = ctypes.CDLL("/opt/axon/libaxon_pjrt.so")
        lib.axon_start_nrt_profile.argtypes = [
            ctypes.POINTER(ctypes.c_int64),
            ctypes.c_size_t,
        ]
        lib.axon_start_nrt_profile.restype = ctypes.c_int64
        lib.axon_stop_nrt_profile.argtypes = [ctypes.c_char_p]
        lib.axon_stop_nrt_profile.restype = ctypes.c_int64
    except Exception:
        return

    @contextlib.contextmanager
    def _hook(output_dir, device_ids):
        import jax

        jax.devices()
        ids = (
            (ctypes.c_int64 * len(device_ids))(*device_ids) if device_ids else None
        )
        rc = lib.axon_start_nrt_profile(ids, len(device_ids) if device_ids else 0)
        if rc != 0:
            raise RuntimeError(f"axon_start_nrt_profile rc={rc}")
        try:
            yield
        finally:
            n = lib.axon_stop_nrt_profile(str(output_dir).encode())
            print(f"profile: {n} file(s) written to {output_dir}")

    mod = types.ModuleType("antenv.axon_hooks")
    mod.get_axon_ntff_profile_hook = lambda: _hook
    mod.set_axon_ntff_profile_hook = lambda h: None
    sys.modules["antenv.axon_hooks"] = mod


def _split_multi_waits(nc):
    """This container's walrus only encodes one sem wait per CTRL-class
    instruction; hoist extra waits onto dedicated single-wait NoOps."""
    idx = 0
    for fn in nc.m.functions:
        for bb in fn.blocks:
            new = []
            for inst in bb.instructions:
                si = inst.sync_info
                if si is not None and len(si.on_wait) > 1:
                    waits = list(si.on_wait)
                    for w in waits[:-1]:
                        c = mybir.InstNoOp(name=f"wsplit-{idx}", ins=[], outs=[])
                        idx += 1
                        c.engine = inst.engine
                        c.sync_info = mybir.SyncInfo(on_wait=[w], on_update=[])
                        new.append(c)
                    si.on_wait = [waits[-1]]
                new.append(inst)
            bb.instructions = new


def _token_chunks(C):
    """Split C (multiple of 128) into chunks <=512, each a multiple of 128,
    preferring >=256 (float32r full-rate threshold)."""
    n = max(1, -(-C // 512))
    tiles = C // P
    base, rem = divmod(tiles, n)
    sizes = [(base + (1 if i < rem else 0)) * P for i in range(n)]
    return [s for s in sizes if s > 0]


def _build_bass(caps):
    F32 = mybir.dt.float32
    F32R = mybir.dt.float32r
    KT = EMB // P  # contraction tiles (8)
    HT = HID // P  # hidden row-blocks (8)

    nc = bass.Bass()
    xt_d, g_d, w1_d, w2_d, wc_d, yt_d = [], [], [], [], [], []
    for e in range(EPC):
        C = caps[e]
        xt_d.append(nc.declare_dram_parameter(f"xt{e}", [EMB, C], F32R, isOutput=False))
        g_d.append(nc.declare_dram_parameter(f"g{e}", [P, C], F32, isOutput=False))
        w1_d.append(
            nc.declare_dram_parameter(f"w1_{e}", [EMB, HID], F32R, isOutput=False)
        )
        w2_d.append(
            nc.declare_dram_parameter(f"w2_{e}", [EMB, HID], F32R, isOutput=False)
        )
        wc_d.append(
            nc.declare_dram_parameter(f"wc_{e}", [HID, EMB], F32R, isOutput=False)
        )
        yt_d.append(nc.declare_dram_parameter(f"yt{e}", [EMB, C], F32, isOutput=True))

    with tile.TileContext(nc) as tc:
        with (
            tc.tile_pool(name="xt", bufs=2) as xt_pool,
            tc.tile_pool(name="ht", bufs=1) as ht_pool,
            tc.tile_pool(name="g", bufs=2) as g_pool,
            tc.tile_pool(name="w12", bufs=4) as w12_pool,
            tc.tile_pool(name="wc", bufs=4) as wc_pool,
            tc.tile_pool(name="s", bufs=3) as s_pool,
            tc.tile_pool(name="y", bufs=3) as y_pool,
            tc.tile_pool(name="psA", bufs=2, space="PSUM") as psA,
            tc.tile_pool(name="psB", bufs=2, space="PSUM") as psB,
        ):
            for e in range(EPC):
                C = caps[e]
                chunks = _token_chunks(C)
                # h=0 weights first: they + the first xt chunk gate the
                # first matmul, so they must not queue behind bulk input DMA.
                w1t0 = w12_pool.tile([P, KT, P], mybir.dt.float32r, tag="w1")
                w2t0 = w12_pool.tile([P, KT, P], mybir.dt.float32r, tag="w2")
                nc.sync.dma_start(
                    w1t0[:], w1_d[e][:, 0:P].rearrange("(k p) q -> p k q", p=P)
                )
                xt_sb = xt_pool.tile([P, KT, C], mybir.dt.float32r, tag="xt")
                xt_view = xt_d[e].rearrange("(k p) c -> p k c", p=P)
                c0 = 0
                for cs in chunks:
                    for k in range(KT):
                        nc.sync.dma_start(
                            xt_sb[:, k, c0 : c0 + cs],
                            xt_view[:, k, c0 : c0 + cs],
                        )
                    if c0 == 0:
                        nc.sync.dma_start(
                            w2t0[:],
                            w2_d[e][:, 0:P].rearrange("(k p) q -> p k q", p=P),
                        )
                    c0 += cs
                g_sb = g_pool.tile([P, C], mybir.dt.float32, tag="g")
                nc.sync.dma_start(g_sb[:], g_d[e][:])
                ht_sb = ht_pool.tile([P, HT, C], mybir.dt.float32r, tag="ht")

                # Phase A: ht = silu(W1.T @ Xt) * (W2.T @ Xt)
                for h in range(HT):
                    if h == 0:
                        w1t, w2t = w1t0, w2t0
                    else:
                        w1t = w12_pool.tile([P, KT, P], mybir.dt.float32r, tag="w1")
                        w2t = w12_pool.tile([P, KT, P], mybir.dt.float32r, tag="w2")
                        nc.sync.dma_start(
                            w1t[:],
                            w1_d[e][:, h * P : (h + 1) * P].rearrange(
                                "(k p) q -> p k q", p=P
                            ),
                        )
                        nc.sync.dma_start(
                            w2t[:],
                            w2_d[e][:, h * P : (h + 1) * P].rearrange(
                                "(k p) q -> p k q", p=P
                            ),
                        )
                    c0 = 0
                    for cs in chunks:
                        ps1 = psA.tile([P, cs], mybir.dt.float32, tag="ps1")
                        ps2 = psA.tile([P, cs], mybir.dt.float32, tag="ps2")
                        for k in range(KT):
                            nc.tensor.matmul(
                                ps1[:],
                                w1t[:, k, :],
                                xt_sb[:, k, c0 : c0 + cs],
                                start=(k == 0),
                                stop=(k == KT - 1),
                            )
                        for k in range(KT):
                            nc.tensor.matmul(
                                ps2[:],
                                w2t[:, k, :],
                                xt_sb[:, k, c0 : c0 + cs],
                                start=(k == 0),
                                stop=(k == KT - 1),
                            )
                        s_sb = s_pool.tile([P, 512], mybir.dt.float32, tag="s")
                        nc.scalar.activation(
                            s_sb[:, :cs],
                            ps1[:],
                            mybir.ActivationFunctionType.Silu,
                        )
                        nc.vector.tensor_mul(
                            ht_sb[:, h, c0 : c0 + cs], s_sb[:, :cs], ps2[:]
                        )
                        c0 += cs

                # Phase B: yt = (WC.T @ ht) * gate
                for d in range(HT):
                    wct = wc_pool.tile([P, HT, P], mybir.dt.float32r, tag="wc")
                    nc.sync.dma_start(
                        wct[:],
                        wc_d[e][:, d * P : (d + 1) * P].rearrange(
                            "(k p) q -> p k q", p=P
                        ),
                    )
                    c0 = 0
                    for cs in chunks:
                        psy = psB.tile([P, cs], mybir.dt.float32, tag="psy")
                        for h in range(HT):
                            nc.tensor.matmul(
                                psy[:],
                                wct[:, h, :],
                                ht_sb[:, h, c0 : c0 + cs],
                                start=(h == 0),
                                stop=(h == HT - 1),
                            )
                        y_sb = y_pool.tile([P, 512], mybir.dt.float32, tag="y")
                        nc.vector.tensor_mul(
                            y_sb[:, :cs], psy[:], g_sb[:, c0 : c0 + cs]
                        )
                        nc.sync.dma_start(
                            yt_d[e][d * P : (d + 1) * P, c0 : c0 + cs],
                            y_sb[:, :cs],
                        )
                        c0 += cs

    _split_multi_waits(nc)
    return nc


def kernel(x, w_gate, w1, w2, wc):
    trace = bool(int(os.environ.get("BASS_MOE_TRACE", "0")))
    if trace:
        _install_profile_shim()

    import concourse.bass_utils as bass_utils

    bass_utils.upload_artifacts = lambda tmpdir: f"local://{tmpdir}"

    x = np.asarray(x, dtype=np.float32)
    w_gate = np.asarray(w_gate, dtype=np.float32)
    w1 = np.asarray(w1, dtype=np.float32)
    w2 = np.asarray(w2, dtype=np.float32)
    wc = np.asarray(wc, dtype=np.float32)

    b, s, d = x.shape
    xf = x.reshape(-1, d)
    n = xf.shape[0]

    # ---- Router on host (float64: stable ranking + gate values) ----
    logits = xf.astype(np.float64) @ w_gate.astype(np.float64)
    mx = logits.max(axis=1, keepdims=True)
    p = np.exp(logits - mx)
    p /= p.sum(axis=1, keepdims=True)
    top = np.argpartition(-logits, TOPK, axis=1)[:, :TOPK]  # top-2 ids (unordered)

    sel_tok = []  # per expert: token indices
    sel_gate = []  # per expert: gate values
    flat_e = top.ravel()
    flat_t = np.repeat(np.arange(n), TOPK)
    order = np.argsort(flat_e, kind="stable")
    se, st = flat_e[order], flat_t[order]
    bounds = np.searchsorted(se, np.arange(E + 1))
    counts = np.diff(bounds)
    for e in range(E):
        toks = st[bounds[e] : bounds[e + 1]]
        sel_tok.append(toks)
        sel_gate.append(p[toks, e].astype(np.float32))

    # ---- Slot assignment: biggest experts in slot 0, smallest in slot 1,
    # so each slot's uniform capacity hugs its experts' actual counts ----
    rank = np.argsort(-counts, kind="stable")
    slot_experts = [
        [int(rank[core + j * NCORES]) for j in range(EPC)] for core in range(NCORES)
    ]
    caps = []
    for j in range(EPC):
        cmax = max(counts[slot_experts[core][j]] for core in range(NCORES))
        caps.append(max(P, int(-(-cmax // P) * P)))

    # ---- Build per-core input maps (expert-parallel: 2 experts/core) ----
    in_maps = []
    for core in range(NCORES):
        m = {}
        for j in range(EPC):
            e = slot_experts[core][j]
            C = caps[j]
            toks = sel_tok[e]
            xt = np.zeros((EMB, C), dtype=np.float32)
            xt[:, : len(toks)] = xf[toks].T
            g = np.zeros((C,), dtype=np.float32)
            g[: len(toks)] = sel_gate[e]
            m[f"xt{j}"] = xt
            m[f"g{j}"] = np.broadcast_to(g, (P, C)).copy()
            m[f"w1_{j}"] = np.ascontiguousarray(w1[e])
            m[f"w2_{j}"] = np.ascontiguousarray(w2[e])
            m[f"wc_{j}"] = np.ascontiguousarray(wc[e])
        in_maps.append(m)

    nc = _build_bass(caps)
    res = bass_utils.run_bass_kernel_spmd(
        nc, in_maps, list(range(NCORES)), trace=trace
    )
    if trace:
        kernel.last_exec_time_ns = res.exec_time_ns
        kernel.last_trace = (
            res.instructions_and_trace[1] if res.instructions_and_trace else None
        )

    # ---- Scatter-add back to token order ----
    out = np.zeros((n, d), dtype=np.float32)
    for core in range(NCORES):
        for j in range(EPC):
            e = slot_experts[core][j]
            toks = sel_tok[e]
            yt = res.results[core][f"yt{j}"]
            out[toks] += yt[:, : len(toks)].T
    return out.reshape(b, s, d)


# revision 11
# speedup vs baseline: 1.0363x; 1.0013x over previous
"""MoE (top-2 of 16 experts, SwiGLU MLP) kernel for 8 Trainium2 NeuronCores.

Strategy (expert-parallel, per sharding hint):
  - Host: router (x @ w_gate -> softmax -> top-2) computed in float64,
    tokens gathered per expert ("all-to-all"). Experts are ranked by
    token count: the 8 largest go in core slot 0, the 8 smallest in
    slot 1, and each slot gets a uniform capacity (its max count,
    rounded up to a multiple of 128) so padding waste stays small.
  - Device (SPMD over 8 cores, 2 experts/core): per expert
        ht = silu(W1e.T @ Xt) * (W2e.T @ Xt)     [feature-major layout]
        yt = (WCe.T @ ht) * gate_row
    All matmuls run as float32r (full PE rate, ~1e-4 rel err),
    accumulating fp32 in PSUM.
  - Host: scatter-add per-expert outputs back to token order.
"""

import contextlib
import ctypes
import os
import sys
import types

sys.path.insert(0, "/opt/trn_rl_repo")

import numpy as np

import concourse.bass as bass
import concourse.mybir as mybir
import concourse.tile as tile

EMB = 1024
HID = 1024
E = 16
TOPK = 2
NCORES = 8
EPC = E // NCORES  # experts per core
P = 128
F32 = None  # set after mybir import below
F32R = None


def _install_profile_shim():
    """Register the axon NTFF profiling hook (missing antenv.axon_hooks in
    this image) so run_bass_kernel_spmd(trace=True) can measure HW time."""
    if "antenv.axon_hooks" in sys.modules:
        return
    try:
        lib = ctypes.CDLL("/opt/axon/libaxon_pjrt.so")
        lib.axon_start_nrt_profile.argtypes = [
            ctypes.POINTER(ctypes.c_int64),
            ctypes.c_size_t,
        ]
        lib.axon_start_nrt_profile.restype = ctypes.c_int64
        lib.axon_stop_nrt_profile.argtypes = [ctypes.c_char_p]
        lib.axon_stop_nrt_profile.restype = ctypes.c_int64
    except Exception:
        return

    @contextlib.contextmanager
    def _hook(output_dir, device_ids):
        import jax

        jax.devices()
        ids = (
            (ctypes.c_int64 * len(device_ids))(*device_ids) if device_ids else None
        )
        rc = lib.axon_start_nrt_profile(ids, len(device_ids) if device_ids else 0)
        if rc != 0:
            raise RuntimeError(f"axon_start_nrt_profile rc={rc}")
        try:
            yield
        finally:
            n = lib.axon_stop_nrt_profile(str(output_dir).encode())
            print(f"profile: {n} file(s) written to {output_dir}")

    mod = types.ModuleType("antenv.axon_hooks")
    mod.get_axon_ntff_profile_hook = lambda: _hook
    mod.set_axon_ntff_profile_hook = lambda h: None
    sys.modules["antenv.axon_hooks"] = mod


def _split_multi_waits(nc):
    """This container's walrus only encodes one sem wait per CTRL-class
    instruction; hoist extra waits onto dedicated single-wait NoOps."""
    idx = 0
    for fn in nc.m.functions:
        for bb in fn.blocks:
            new = []
            for inst in bb.instructions:
                si = inst.sync_info
                if si is not None and len(si.on_wait) > 1:
                    waits = list(si.on_wait)
                    for w in waits[:-1]:
                        c = mybir.InstNoOp(name=f"wsplit-{idx}", ins=[], outs=[])
                        idx += 1
                        c.engine = inst.engine
                        c.sync_info = mybir.SyncInfo(on_wait=[w], on_update=[])
                        new.append(c)
                    si.on_wait = [waits[-1]]
                new.append(inst)
            bb.instructions = new


def _token_chunks(C):
    """Split C (multiple of 128) into chunks <=512, each a multiple of 128,
    preferring >=256 (float32r full-rate threshold)."""
    n = max(1, -(-C // 512))
    tiles = C // P
    base, rem = divmod(tiles, n)
    sizes = [(base + (1 if i < rem else 0)) * P for i in range(n)]
    return [s for s in sizes if s > 0]


def _build_bass(caps):
    F32 = mybir.dt.float32
    F32R = mybir.dt.float32r
    KT = EMB // P  # contraction tiles (8)
    HT = HID // P  # hidden row-blocks (8)

    nc = bass.Bass()
    xt_d, g_d, w1_d, w2_d, wc_d, yt_d = [], [], [], [], [], []
    for e in range(EPC):
        C = caps[e]
        xt_d.append(nc.declare_dram_parameter(f"xt{e}", [EMB, C], F32R, isOutput=False))
        g_d.append(nc.declare_dram_parameter(f"g{e}", [P, C], F32, isOutput=False))
        w1_d.append(
            nc.declare_dram_parameter(f"w1_{e}", [EMB, HID], F32R, isOutput=False)
        )
        w2_d.append(
            nc.declare_dram_parameter(f"w2_{e}", [EMB, HID], F32R, isOutput=False)
        )
        wc_d.append(
            nc.declare_dram_parameter(f"wc_{e}", [HID, EMB], F32R, isOutput=False)
        )
        yt_d.append(nc.declare_dram_parameter(f"yt{e}", [EMB, C], F32, isOutput=True))

    with tile.TileContext(nc) as tc:
        with (
            tc.tile_pool(name="xt", bufs=2) as xt_pool,
            tc.tile_pool(name="ht", bufs=1) as ht_pool,
            tc.tile_pool(name="g", bufs=2) as g_pool,
            tc.tile_pool(name="w12", bufs=4) as w12_pool,
            tc.tile_pool(name="wc", bufs=4) as wc_pool,
            tc.tile_pool(name="s", bufs=3) as s_pool,
            tc.tile_pool(name="y", bufs=3) as y_pool,
            tc.tile_pool(name="psA", bufs=2, space="PSUM") as psA,
            tc.tile_pool(name="psB", bufs=2, space="PSUM") as psB,
        ):
            for e in range(EPC):
                C = caps[e]
                chunks = _token_chunks(C)
                # h=0 weights first: they + the first xt chunk gate the
                # first matmul, so they must not queue behind bulk input DMA.
                w1t0 = w12_pool.tile([P, KT, P], mybir.dt.float32r, tag="w1")
                w2t0 = w12_pool.tile([P, KT, P], mybir.dt.float32r, tag="w2")
                nc.sync.dma_start(
                    w1t0[:], w1_d[e][:, 0:P].rearrange("(k p) q -> p k q", p=P)
                )
                xt_sb = xt_pool.tile([P, KT, C], mybir.dt.float32r, tag="xt")
                xt_view = xt_d[e].rearrange("(k p) c -> p k c", p=P)
                c0 = 0
                for cs in chunks:
                    for k in range(KT):
                        nc.sync.dma_start(
                            xt_sb[:, k, c0 : c0 + cs],
                            xt_view[:, k, c0 : c0 + cs],
                        )
                    if c0 == 0:
                        nc.sync.dma_start(
                            w2t0[:],
                            w2_d[e][:, 0:P].rearrange("(k p) q -> p k q", p=P),
                        )
                    c0 += cs
                g_sb = g_pool.tile([P, C], mybir.dt.float32, tag="g")
                nc.sync.dma_start(g_sb[:], g_d[e][:])
                ht_sb = ht_pool.tile([P, HT, C], mybir.dt.float32r, tag="ht")

                # Phase A: ht = silu(W1.T @ Xt) * (W2.T @ Xt)
                for h in range(HT):
                    if h == 0:
                        w1t, w2t = w1t0, w2t0
                    else:
                        w1t = w12_pool.tile([P, KT, P], mybir.dt.float32r, tag="w1")
                        w2t = w12_pool.tile([P, KT, P], mybir.dt.float32r, tag="w2")
                        nc.sync.dma_start(
                            w1t[:],
                            w1_d[e][:, h * P : (h + 1) * P].rearrange(
                                "(k p) q -> p k q", p=P
                            ),
                        )
                        nc.sync.dma_start(
                            w2t[:],
                            w2_d[e][:, h * P : (h + 1) * P].rearrange(
                                "(k p) q -> p k q", p=P
                            ),
                        )
                    c0 = 0
                    for cs in chunks:
                        ps1 = psA.tile([P, cs], mybir.dt.float32, tag="ps1")
                        ps2 = psA.tile([P, cs], mybir.dt.float32, tag="ps2")
                        for k in range(KT):
                            nc.tensor.matmul(
                                ps1[:],
                                w1t[:, k, :],
                                xt_sb[:, k, c0 : c0 + cs],
                                start=(k == 0),
                                stop=(k == KT - 1),
                            )
                        for k in range(KT):
                            nc.tensor.matmul(
                                ps2[:],
                                w2t[:, k, :],
                                xt_sb[:, k, c0 : c0 + cs],
                                start=(k == 0),
                                stop=(k == KT - 1),
                            )
                        s_sb = s_pool.tile([P, 512], mybir.dt.float32, tag="s")
                        nc.scalar.activation(
                            s_sb[:, :cs],
                            ps1[:],
                            mybir.ActivationFunctionType.Silu,
                        )
                        nc.vector.tensor_mul(
                            ht_sb[:, h, c0 : c0 + cs], s_sb[:, :cs], ps2[:]
                        )
                        c0 += cs

                # Phase B: yt = (WC.T @ ht) * gate
                for d in range(HT):
                    wct = wc_pool.tile([P, HT, P], mybir.dt.float32r, tag="wc")
                    nc.sync.dma_start(
                        wct[:],
                        wc_d[e][:, d * P : (d + 1) * P].rearrange(
                            "(k p) q -> p k q", p=P
                        ),
                    )
                    c0 = 0
                    for cs in chunks:
                        psy = psB.tile([P, cs], mybir.dt.float32, tag="psy")
                        for h in range(HT):
                            nc.tensor.matmul(
                                psy[:],
                                wct[:, h, :],
                                ht_sb[:, h, c0 : c0 + cs],
                                start=(h == 0),
                                stop=(h == HT - 1),
                            )
                        y_sb = y_pool.tile([P, 512], mybir.dt.float32, tag="y")
                        nc.vector.tensor_mul(
                            y_sb[:, :cs], psy[:], g_sb[:, c0 : c0 + cs]
                        )
                        nc.sync.dma_start(
                            yt_d[e][d * P : (d + 1) * P, c0 : c0 + cs],
                            y_sb[:, :cs],
                        )
                        c0 += cs

    _split_multi_waits(nc)
    return nc


def kernel(x, w_gate, w1, w2, wc):
    trace = bool(int(os.environ.get("BASS_MOE_TRACE", "0")))
    if trace:
        _install_profile_shim()

    import concourse.bass_utils as bass_utils

    bass_utils.upload_artifacts = lambda tmpdir: f"local://{tmpdir}"

    x = np.asarray(x, dtype=np.float32)
    w_gate = np.asarray(w_gate, dtype=np.float32)
    w1 = np.asarray(w1, dtype=np.float32)
    w2 = np.asarray(w2, dtype=np.float32)
    wc = np.asarray(wc, dtype=np.float32)

    b, s, d = x.shape
    xf = x.reshape(-1, d)
    n = xf.shape[0]

    # ---- Router on host (float64: stable ranking + gate values) ----
    logits = xf.astype(np.float64) @ w_gate.astype(np.float64)
    mx = logits.max(axis=1, keepdims=True)
    p = np.exp(logits - mx)
    p /= p.sum(axis=1, keepdims=True)
    top = np.argpartition(-logits, TOPK, axis=1)[:, :TOPK]  # top-2 ids (unordered)

    sel_tok = []  # per expert: token indices
    sel_gate = []  # per expert: gate values
    flat_e = top.ravel()
    flat_t = np.repeat(np.arange(n), TOPK)
    order = np.argsort(flat_e, kind="stable")
    se, st = flat_e[order], flat_t[order]
    bounds = np.searchsorted(se, np.arange(E + 1))
    counts = np.diff(bounds)
    for e in range(E):
        toks = st[bounds[e] : bounds[e + 1]]
        sel_tok.append(toks)
        sel_gate.append(p[toks, e].astype(np.float32))

    # ---- Slot assignment: biggest experts in slot 0, smallest in slot 1,
    # so each slot's uniform capacity hugs its experts' actual counts ----
    rank = np.argsort(-counts, kind="stable")
    slot_experts = [
        [int(rank[core + j * NCORES]) for j in range(EPC)] for core in range(NCORES)
    ]
    caps = []
    for j in range(EPC):
        cmax = max(counts[slot_experts[core][j]] for core in range(NCORES))
        caps.append(max(P, int(-(-cmax // P) * P)))

    # ---- Build per-core input maps (expert-parallel: 2 experts/core) ----
    in_maps = []
    for core in range(NCORES):
        m = {}
        for j in range(EPC):
            e = slot_experts[core][j]
            C = caps[j]
            toks = sel_tok[e]
            xt = np.zeros((EMB, C), dtype=np.float32)
            xt[:, : len(toks)] = xf[toks].T
            g = np.zeros((C,), dtype=np.float32)
            g[: len(toks)] = sel_gate[e]
            m[f"xt{j}"] = xt
            m[f"g{j}"] = np.broadcast_to(g, (P, C)).copy()
            m[f"w1_{j}"] = np.ascontiguousarray(w1[e])
            m[f"w2_{j}"] = np.ascontiguousarray(w2[e])
            m[f"wc_{j}"] = np.ascontiguousarray(wc[e])
        in_maps.append(m)

    nc = _build_bass(caps)
    res = bass_utils.run_bass_kernel_spmd(
        nc, in_maps, list(range(NCORES)), trace=trace
    )
    if trace:
        kernel.last_exec_time_ns = res.exec_time_ns
        kernel.last_trace = (
            res.instructions_and_trace[1] if res.instructions_and_trace else None
        )

    # ---- Scatter-add back to token order ----
    out = np.zeros((n, d), dtype=np.float32)
    for core in range(NCORES):
        for j in range(EPC):
            e = slot_experts[core][j]
            toks = sel_tok[e]
            yt = res.results[core][f"yt{j}"]
            out[toks] += yt[:, : len(toks)].T
    return out.reshape(b, s, d)
